# revision 30
# baseline (speedup 1.0000x reference)
"""MoE routing kernel for Trainium2 (8 NeuronCores, SPMD data-parallel).

Computes, for x [4, 4096, 4096] f32, proto_k [64, 4096] f32, gate [64] f32:
    logits = relu(x @ proto_k.T / sqrt(4096) - gate)        # [B, S, 64]
    routing_weights, selected_experts = top_k(logits, k=8)  # [B, S, 8] each

Sharding: tokens (B*S = 16384) are split evenly across 8 cores (2048 each).
proto_k / gate are replicated. No collectives needed.

Numerics: the matmul runs as a 3-term fp16 hi/lo split (x = xh + xl,
proto = ph + pl, logits = xh@ph + xh@pl + xl@ph, dropping xl@pl ~ 2^-22).
The residuals are pre-scaled by 2^11 on the host so they stay in fp16's
normal range, accumulated in a second PSUM bank, and recombined as
hi + 2^-11 * lo on the DVE.  Validated: bit-noise-level agreement with the
fp32 reference (max logit perturbation ~4e-8, zero top-8 index flips),
while streaming the PE at fp16 rate (1 cycle/row, 3 passes) instead of
fp32's 4 cycles/row with serialized weight loads.

Per-core device program:
  - x shard is split/transposed on the host to xh/xl [4096, 2048] fp16 so
    every DMA is contiguous and the contraction dim rides SBUF partitions.
  - logits accumulate with experts on partitions: per 128-wide hidden chunk,
    3 matmuls into 2 PSUM banks ([64, 512] per 512-token group).
  - DVE recombines hi + 2^-11*lo; ScalarE applies relu(acc/64 - gate).
  - TensorE transposes [64, 128] tiles -> [128 tokens, 64 experts] PSUM.
  - DVE Max8/MaxIndex emit top-8 values (descending) + indices per token.
  - Outputs pack as [128, 16*8] tiles, unscrambled on the host.
"""

import numpy as np

HIDDEN = 4096
NUM_EXPERTS = 64
TOP_K = 8
N_CORES = 8
TOKENS = 4 * 4096
T_CORE = TOKENS // N_CORES          # 2048 tokens per core
N_CHUNK = HIDDEN // 128             # 32 contraction chunks
GROUPS_PER_PASS = 2                 # 512-token groups accumulated per pass
N_PASS = T_CORE // (512 * GROUPS_PER_PASS)
N_SUB = T_CORE // 128               # 16 output sub-tiles of 128 tokens
LO_SCALE = np.float32(2.0 ** 11)
LO_UNSCALE = 2.0 ** -11

_PROGRAM = None


def _split_multi_waits(nc):
    """walrus in this container rejects instructions carrying more sync waits
    than their ISA struct holds (setupSyncWait: 'Too many sync wait
    commands'); Drain takes one, S3_LW (matmul weight-load) ~two.  Normalize
    every instruction to a single wait by hoisting extras onto same-engine
    NOPs inserted immediately before the owner."""
    import bass_rust

    inserts = {}  # owner inst name -> list of wait-nop instructions
    for f in nc.m.functions:
        for bb in f.blocks:
            for inst in bb.instructions:
                si = inst.sync_info
                if si is None or len(si.on_wait) <= 1:
                    continue
                conds = list(si.on_wait)
                si.on_wait = conds[:1]
                eng = nc.engines[inst.engine]
                new_insts = []
                for w in conds[1:]:
                    nop = eng.nop(hint="split_wait")
                    nop.ins.sync_info = bass_rust.SyncInfo(
                        on_wait=[w], on_update=[]
                    )
                    new_insts.append(nop.ins)
                inserts[inst.name] = new_insts
    if not inserts:
        return
    # nop() appended the new instructions to whatever bb was current; strip
    # them from everywhere, then re-insert each right before its owner so
    # the engine observes every wait before executing the instruction.
    appended = {ni.name for nis in inserts.values() for ni in nis}
    for f in nc.m.functions:
        for bb in f.blocks:
            rebuilt = []
            changed = False
            for inst in bb.instructions:
                if inst.name in appended:
                    changed = True
                    continue
                if inst.name in inserts:
                    rebuilt.extend(inserts[inst.name])
                    changed = True
                rebuilt.append(inst)
            if changed:
                bb.instructions = rebuilt


def _build_program():
    import concourse.bass as bass
    import concourse.mybir as mybir
    import concourse.tile as tile

    f32 = mybir.dt.float32
    f16 = mybir.dt.float16
    u32 = mybir.dt.uint32
    E = NUM_EXPERTS

    nc = bass.Bass("TRN2", target_bir_lowering=False, debug=False)

    # xh and xl stacked: xhl[0] = hi, xhl[1] = lo (one DMA fetches both)
    xhl_d = nc.dram_tensor("xhl", [2, HIDDEN, T_CORE], f16, kind="ExternalInput")
    # proto hi|lo packed along expert columns: [:, 0:64] = ph, [:, 64:128] = pl
    phpl_d = nc.dram_tensor("phpl", [HIDDEN, 2 * E], f16, kind="ExternalInput")
    gate_neg = nc.dram_tensor("gate_neg", [E, 1], f32, kind="ExternalInput")
    w_out = nc.dram_tensor("w_out", [128, N_SUB * TOP_K], f32, kind="ExternalOutput")
    i_out = nc.dram_tensor("i_out", [128, N_SUB * TOP_K], u32, kind="ExternalOutput")

    ident_dram = nc.inline_tensor(np.eye(E, dtype=np.float32), name="ident64")

    with tile.TileContext(nc) as tc:
        with (
            tc.tile_pool(name="const", bufs=1) as const_pool,
            tc.tile_pool(name="xa", bufs=8) as x_pool,
            tc.tile_pool(name="acc", bufs=7, space="PSUM") as acc_pool,
            tc.tile_pool(name="tp", bufs=1, space="PSUM") as tp_pool,
            tc.tile_pool(name="lg", bufs=3) as lg_pool,
            tc.tile_pool(name="tk", bufs=3) as tk_pool,
            tc.tile_pool(name="outp", bufs=1) as out_pool,
        ):
            # --- constants ---
            # proto chunks land as [128, c, E]; per-chunk DMAs are contiguous
            # 32 KB and let the first matmuls start early.
            # weights ride the (otherwise idle) gpsimd SWDGE ring so neither
            # the x stream (sync ring) nor the epilogue traffic (scalar
            # ring) queues behind their 32 triggers.
            phpl_sb = const_pool.tile([128, N_CHUNK * 2 * E], f16)
            for c in range(N_CHUNK):
                nc.gpsimd.dma_start(
                    phpl_sb[:, c * 2 * E:(c + 1) * 2 * E],
                    phpl_d[c * 128:(c + 1) * 128, :],
                )
            gate_sb = const_pool.tile([E, 1], f32)
            nc.scalar.dma_start(gate_sb[:], gate_neg[:])
            ident_sb = const_pool.tile([E, E], f32)
            nc.scalar.dma_start(ident_sb[:], ident_dram[:])

            vals_sb = out_pool.tile([128, N_SUB * TOP_K], f32)
            idx_sb = out_pool.tile([128, N_SUB * TOP_K], u32)

            for p in range(N_PASS):
                tpp = GROUPS_PER_PASS * 512
                t0 = p * tpp
                # a = xh @ [ph|pl]: rows 0:64 main term, 64:128 lo (2^11)
                # b = xl @ [ph|pl]: rows 0:64 lo (2^11), 64:128 llo (2^22)
                a_accs = [
                    acc_pool.tile([128, 512], f32, name=f"a_p{p}g{g}", tag="acc")
                    for g in range(GROUPS_PER_PASS)
                ]
                b_accs = [
                    acc_pool.tile([128, 512], f32, name=f"b_p{p}g{g}", tag="acc")
                    for g in range(GROUPS_PER_PASS)
                ]
                for c in range(N_CHUNK):
                    # one HWDGE DMA per chunk fetches hi and lo halves;
                    # alternate between the two HWDGE rings (SP / ACT) so
                    # trigger issue is never the bottleneck
                    x_t = x_pool.tile([128, 2, tpp], f16, name="x_t", tag="xt")
                    ring = nc.sync if c % 2 == 0 else nc.scalar
                    ring.dma_start(
                        x_t[:], xhl_d[:, c * 128:(c + 1) * 128, t0:t0 + tpp]
                        .rearrange("s p t -> p s t")
                    )
                    first, last = (c == 0), (c == N_CHUNK - 1)
                    pc = slice(c * 2 * E, (c + 1) * 2 * E)
                    for g in range(GROUPS_PER_PASS):
                        ts = slice(g * 512, (g + 1) * 512)
                        nc.tensor.matmul(
                            a_accs[g][:], phpl_sb[:, pc], x_t[:, 0, ts],
                            start=first, stop=last,
                        )
                        nc.tensor.matmul(
                            b_accs[g][:], phpl_sb[:, pc], x_t[:, 1, ts],
                            start=first, stop=last,
                        )
                for g in range(GROUPS_PER_PASS):
                    # comb = a[0:64] + 2^-11*(a[64:128] + b[0:64] + 2^-11*b[64:128])
                    # DVE reads at most one PSUM input per op, so `a` is
                    # staged through SBUF (which also releases its PSUM bank
                    # for the next pass early).  The reads of the [64:128]
                    # halves into 0:64-partition outputs are cross-partition
                    # APs — verified exact on hardware.
                    a_sb = lg_pool.tile([128, 512], f32, name="a_sb")
                    nc.vector.tensor_copy(a_sb[:], a_accs[g][:])
                    u = lg_pool.tile([E, 512], f32, name="u")
                    nc.vector.scalar_tensor_tensor(
                        u[:], b_accs[g][0:E, :], 1.0, a_sb[E:2 * E, :],
                        bass.mybir.AluOpType.mult, bass.mybir.AluOpType.add,
                    )
                    v = lg_pool.tile([E, 512], f32, name="v")
                    nc.vector.scalar_tensor_tensor(
                        v[:], b_accs[g][E:2 * E, :], LO_UNSCALE, u[:],
                        bass.mybir.AluOpType.mult, bass.mybir.AluOpType.add,
                    )
                    comb = lg_pool.tile([E, 512], f32, name="comb")
                    nc.vector.scalar_tensor_tensor(
                        comb[:], v[:], LO_UNSCALE, a_sb[0:E, :],
                        bass.mybir.AluOpType.mult, bass.mybir.AluOpType.add,
                    )
                    # relu(acc/64 - gate)  (ScalarE, SBUF -> SBUF)
                    logits = lg_pool.tile([E, 512], f32, name="logits")
                    nc.scalar.activation(
                        logits[:], comb[:],
                        bass.mybir.ActivationFunctionType.Relu,
                        bias=gate_sb[:], scale=1.0 / 64.0,
                    )
                    # transpose to [128 tokens, 64 experts] x 4 sub-tiles
                    tk_psum = tp_pool.tile([128, 4 * E], f32, name="tk_psum")
                    for j in range(4):
                        nc.tensor.transpose(
                            tk_psum[:, j * E:(j + 1) * E],
                            logits[:, j * 128:(j + 1) * 128],
                            ident_sb[:],
                        )
                    tk_sb = tk_pool.tile([128, 4 * E], f32, name="tk_sb")
                    nc.vector.tensor_copy(tk_sb[:], tk_psum[:])
                    gg = p * GROUPS_PER_PASS + g
                    for j in range(4):
                        s = gg * 4 + j
                        nc.vector.max(
                            vals_sb[:, s * TOP_K:(s + 1) * TOP_K],
                            tk_sb[:, j * E:(j + 1) * E],
                        )
                        nc.vector.max_index(
                            idx_sb[:, s * TOP_K:(s + 1) * TOP_K],
                            vals_sb[:, s * TOP_K:(s + 1) * TOP_K],
                            tk_sb[:, j * E:(j + 1) * E],
                        )
                # flush this pass's outputs so only the last pass's epilogue
                # sits in the kernel tail
                os_ = slice(p * GROUPS_PER_PASS * 4 * TOP_K,
                            (p + 1) * GROUPS_PER_PASS * 4 * TOP_K)
                nc.scalar.dma_start(w_out[:, os_], vals_sb[:, os_])
                nc.scalar.dma_start(i_out[:, os_], idx_sb[:, os_])

    _split_multi_waits(nc)
    return nc


def _get_program():
    global _PROGRAM
    if _PROGRAM is None:
        _PROGRAM = _build_program()
    return _PROGRAM


def _make_in_maps(x, proto_k, gate):
    xf = np.ascontiguousarray(x, dtype=np.float32).reshape(TOKENS, HIDDEN)
    proto = np.asarray(proto_k, dtype=np.float32)
    ph = proto.astype(np.float16)
    pl = ((proto - ph.astype(np.float32)) * LO_SCALE).astype(np.float16)
    phpl = np.concatenate([ph.T, pl.T], axis=1)           # [4096, 128] f16
    gate_neg = np.ascontiguousarray(
        -np.asarray(gate, dtype=np.float32).reshape(NUM_EXPERTS, 1)
    )
    in_maps = []
    for c in range(N_CORES):
        shard_t = xf[c * T_CORE:(c + 1) * T_CORE].T       # [4096, 2048] view
        xhl = np.empty((2, HIDDEN, T_CORE), np.float16)
        xhl[0] = shard_t
        xhl[1] = (shard_t - xhl[0].astype(np.float32)) * LO_SCALE
        in_maps.append(
            {"xhl": xhl, "phpl": phpl, "gate_neg": gate_neg}
        )
    return in_maps


def _gather(results):
    w = np.empty((TOKENS, TOP_K), np.float32)
    idx = np.empty((TOKENS, TOP_K), np.int32)
    for c in range(N_CORES):
        wo = results[c]["w_out"]                          # [128, 16*8]
        io = results[c]["i_out"].view(np.int32)
        w[c * T_CORE:(c + 1) * T_CORE] = (
            wo.reshape(128, N_SUB, TOP_K).transpose(1, 0, 2).reshape(T_CORE, TOP_K)
        )
        idx[c * T_CORE:(c + 1) * T_CORE] = (
            io.reshape(128, N_SUB, TOP_K).transpose(1, 0, 2).reshape(T_CORE, TOP_K)
        )
    return w.reshape(4, 4096, TOP_K), idx.reshape(4, 4096, TOP_K)


def run_sharded(in_maps, trace=False, trace_cores=None):
    from concourse.bass_utils import run_bass_kernel_spmd

    nc = _get_program()
    return run_bass_kernel_spmd(
        nc,
        in_maps,
        core_ids=list(range(N_CORES)),
        trace=trace,
        trace_cores=trace_cores,
    )


def kernel(x, proto_k, gate):
    in_maps = _make_in_maps(x, proto_k, gate)
    res = run_sharded(in_maps, trace=False)
    return _gather(res.results)


# revision 33
# speedup vs baseline: 1.0066x; 1.0066x over previous
"""MoE routing kernel for Trainium2 (8 NeuronCores, SPMD data-parallel).

Computes, for x [4, 4096, 4096] f32, proto_k [64, 4096] f32, gate [64] f32:
    logits = relu(x @ proto_k.T / sqrt(4096) - gate)        # [B, S, 64]
    routing_weights, selected_experts = top_k(logits, k=8)  # [B, S, 8] each

Sharding: tokens (B*S = 16384) are split evenly across 8 cores (2048 each).
proto_k / gate are replicated. No collectives needed.

Numerics: the matmul runs as a 3-term fp16 hi/lo split (x = xh + xl,
proto = ph + pl, logits = xh@ph + xh@pl + xl@ph, dropping xl@pl ~ 2^-22).
The residuals are pre-scaled by 2^11 on the host so they stay in fp16's
normal range, accumulated in a second PSUM bank, and recombined as
hi + 2^-11 * lo on the DVE.  Validated: bit-noise-level agreement with the
fp32 reference (max logit perturbation ~4e-8, zero top-8 index flips),
while streaming the PE at fp16 rate (1 cycle/row, 3 passes) instead of
fp32's 4 cycles/row with serialized weight loads.

Per-core device program:
  - x shard is split/transposed on the host to xh/xl [4096, 2048] fp16 so
    every DMA is contiguous and the contraction dim rides SBUF partitions.
  - logits accumulate with experts on partitions: per 128-wide hidden chunk,
    3 matmuls into 2 PSUM banks ([64, 512] per 512-token group).
  - DVE recombines hi + 2^-11*lo; ScalarE applies relu(acc/64 - gate).
  - TensorE transposes [64, 128] tiles -> [128 tokens, 64 experts] PSUM.
  - DVE Max8/MaxIndex emit top-8 values (descending) + indices per token.
  - Outputs pack as [128, 16*8] tiles, unscrambled on the host.
"""

import numpy as np

HIDDEN = 4096
NUM_EXPERTS = 64
TOP_K = 8
N_CORES = 8
TOKENS = 4 * 4096
T_CORE = TOKENS // N_CORES          # 2048 tokens per core
N_CHUNK = HIDDEN // 128             # 32 contraction chunks
GROUPS_PER_PASS = 2                 # 512-token groups accumulated per pass
N_PASS = T_CORE // (512 * GROUPS_PER_PASS)
N_SUB = T_CORE // 128               # 16 output sub-tiles of 128 tokens
LO_SCALE = np.float32(2.0 ** 11)
LO_UNSCALE = 2.0 ** -11

_PROGRAM = None


def _split_multi_waits(nc):
    """walrus in this container rejects instructions carrying more sync waits
    than their ISA struct holds (setupSyncWait: 'Too many sync wait
    commands'); Drain takes one, S3_LW (matmul weight-load) ~two.  Normalize
    every instruction to a single wait by hoisting extras onto same-engine
    NOPs inserted immediately before the owner."""
    import bass_rust

    inserts = {}  # owner inst name -> list of wait-nop instructions
    for f in nc.m.functions:
        for bb in f.blocks:
            for inst in bb.instructions:
                si = inst.sync_info
                if si is None or len(si.on_wait) <= 1:
                    continue
                conds = list(si.on_wait)
                si.on_wait = conds[:1]
                eng = nc.engines[inst.engine]
                new_insts = []
                for w in conds[1:]:
                    nop = eng.nop(hint="split_wait")
                    nop.ins.sync_info = bass_rust.SyncInfo(
                        on_wait=[w], on_update=[]
                    )
                    new_insts.append(nop.ins)
                inserts[inst.name] = new_insts
    if not inserts:
        return
    # nop() appended the new instructions to whatever bb was current; strip
    # them from everywhere, then re-insert each right before its owner so
    # the engine observes every wait before executing the instruction.
    appended = {ni.name for nis in inserts.values() for ni in nis}
    for f in nc.m.functions:
        for bb in f.blocks:
            rebuilt = []
            changed = False
            for inst in bb.instructions:
                if inst.name in appended:
                    changed = True
                    continue
                if inst.name in inserts:
                    rebuilt.extend(inserts[inst.name])
                    changed = True
                rebuilt.append(inst)
            if changed:
                bb.instructions = rebuilt


def _build_program():
    import concourse.bass as bass
    import concourse.mybir as mybir
    import concourse.tile as tile

    f32 = mybir.dt.float32
    f16 = mybir.dt.float16
    u32 = mybir.dt.uint32
    E = NUM_EXPERTS

    nc = bass.Bass("TRN2", target_bir_lowering=False, debug=False)

    # xh and xl stacked: xhl[0] = hi, xhl[1] = lo (one DMA fetches both)
    xhl_d = nc.dram_tensor("xhl", [2, HIDDEN, T_CORE], f16, kind="ExternalInput")
    # proto hi|lo packed along expert columns: [:, 0:64] = ph, [:, 64:128] = pl
    phpl_d = nc.dram_tensor("phpl", [HIDDEN, 2 * E], f16, kind="ExternalInput")
    gate_neg = nc.dram_tensor("gate_neg", [E, 1], f32, kind="ExternalInput")
    w_out = nc.dram_tensor("w_out", [128, N_SUB * TOP_K], f32, kind="ExternalOutput")
    i_out = nc.dram_tensor("i_out", [128, N_SUB * TOP_K], u32, kind="ExternalOutput")

    ident_dram = nc.inline_tensor(np.eye(E, dtype=np.float32), name="ident64")

    with tile.TileContext(nc) as tc:
        with (
            tc.tile_pool(name="const", bufs=1) as const_pool,
            tc.tile_pool(name="xa", bufs=12) as x_pool,
            tc.tile_pool(name="acc", bufs=7, space="PSUM") as acc_pool,
            tc.tile_pool(name="tp", bufs=1, space="PSUM") as tp_pool,
            tc.tile_pool(name="lg", bufs=3) as lg_pool,
            tc.tile_pool(name="tk", bufs=3) as tk_pool,
            tc.tile_pool(name="outp", bufs=1) as out_pool,
        ):
            # --- constants ---
            # proto chunks land as [128, c, E]; per-chunk DMAs are contiguous
            # 32 KB and let the first matmuls start early.
            # weights ride the (otherwise idle) gpsimd SWDGE ring so neither
            # the x stream (sync ring) nor the epilogue traffic (scalar
            # ring) queues behind their 32 triggers.
            phpl_sb = const_pool.tile([128, N_CHUNK * 2 * E], f16)
            for c in range(N_CHUNK):
                nc.gpsimd.dma_start(
                    phpl_sb[:, c * 2 * E:(c + 1) * 2 * E],
                    phpl_d[c * 128:(c + 1) * 128, :],
                )
            gate_sb = const_pool.tile([E, 1], f32)
            nc.scalar.dma_start(gate_sb[:], gate_neg[:])
            ident_sb = const_pool.tile([E, E], f32)
            nc.scalar.dma_start(ident_sb[:], ident_dram[:])

            vals_sb = out_pool.tile([128, N_SUB * TOP_K], f32)
            idx_sb = out_pool.tile([128, N_SUB * TOP_K], u32)

            for p in range(N_PASS):
                tpp = GROUPS_PER_PASS * 512
                t0 = p * tpp
                # a = xh @ [ph|pl]: rows 0:64 main term, 64:128 lo (2^11)
                # b = xl @ [ph|pl]: rows 0:64 lo (2^11), 64:128 llo (2^22)
                a_accs = [
                    acc_pool.tile([128, 512], f32, name=f"a_p{p}g{g}", tag="acc")
                    for g in range(GROUPS_PER_PASS)
                ]
                b_accs = [
                    acc_pool.tile([128, 512], f32, name=f"b_p{p}g{g}", tag="acc")
                    for g in range(GROUPS_PER_PASS)
                ]
                for c in range(N_CHUNK):
                    # one HWDGE DMA per chunk fetches hi and lo halves;
                    # alternate between the two HWDGE rings (SP / ACT) so
                    # trigger issue is never the bottleneck
                    x_t = x_pool.tile([128, 2, tpp], f16, name="x_t", tag="xt")
                    ring = nc.sync if c % 2 == 0 else nc.scalar
                    ring.dma_start(
                        x_t[:], xhl_d[:, c * 128:(c + 1) * 128, t0:t0 + tpp]
                        .rearrange("s p t -> p s t")
                    )
                    first, last = (c == 0), (c == N_CHUNK - 1)
                    pc = slice(c * 2 * E, (c + 1) * 2 * E)
                    for g in range(GROUPS_PER_PASS):
                        ts = slice(g * 512, (g + 1) * 512)
                        nc.tensor.matmul(
                            a_accs[g][:], phpl_sb[:, pc], x_t[:, 0, ts],
                            start=first, stop=last,
                        )
                        nc.tensor.matmul(
                            b_accs[g][:], phpl_sb[:, pc], x_t[:, 1, ts],
                            start=first, stop=last,
                        )
                for g in range(GROUPS_PER_PASS):
                    # comb = a[0:64] + 2^-11*(a[64:128] + b[0:64] + 2^-11*b[64:128])
                    # DVE reads at most one PSUM input per op, so `a` is
                    # staged through SBUF (which also releases its PSUM bank
                    # for the next pass early).  The reads of the [64:128]
                    # halves into 0:64-partition outputs are cross-partition
                    # APs — verified exact on hardware.
                    a_sb = lg_pool.tile([128, 512], f32, name="a_sb")
                    nc.vector.tensor_copy(a_sb[:], a_accs[g][:])
                    u = lg_pool.tile([E, 512], f32, name="u")
                    nc.vector.scalar_tensor_tensor(
                        u[:], b_accs[g][0:E, :], 1.0, a_sb[E:2 * E, :],
                        bass.mybir.AluOpType.mult, bass.mybir.AluOpType.add,
                    )
                    v = lg_pool.tile([E, 512], f32, name="v")
                    nc.vector.scalar_tensor_tensor(
                        v[:], b_accs[g][E:2 * E, :], LO_UNSCALE, u[:],
                        bass.mybir.AluOpType.mult, bass.mybir.AluOpType.add,
                    )
                    comb = lg_pool.tile([E, 512], f32, name="comb")
                    nc.vector.scalar_tensor_tensor(
                        comb[:], v[:], LO_UNSCALE, a_sb[0:E, :],
                        bass.mybir.AluOpType.mult, bass.mybir.AluOpType.add,
                    )
                    # relu(acc/64 - gate)  (ScalarE, SBUF -> SBUF)
                    logits = lg_pool.tile([E, 512], f32, name="logits")
                    nc.scalar.activation(
                        logits[:], comb[:],
                        bass.mybir.ActivationFunctionType.Relu,
                        bias=gate_sb[:], scale=1.0 / 64.0,
                    )
                    # transpose to [128 tokens, 64 experts] x 4 sub-tiles
                    tk_psum = tp_pool.tile([128, 4 * E], f32, name="tk_psum")
                    for j in range(4):
                        nc.tensor.transpose(
                            tk_psum[:, j * E:(j + 1) * E],
                            logits[:, j * 128:(j + 1) * 128],
                            ident_sb[:],
                        )
                    tk_sb = tk_pool.tile([128, 4 * E], f32, name="tk_sb")
                    nc.vector.tensor_copy(tk_sb[:], tk_psum[:])
                    gg = p * GROUPS_PER_PASS + g
                    for j in range(4):
                        s = gg * 4 + j
                        nc.vector.max(
                            vals_sb[:, s * TOP_K:(s + 1) * TOP_K],
                            tk_sb[:, j * E:(j + 1) * E],
                        )
                        nc.vector.max_index(
                            idx_sb[:, s * TOP_K:(s + 1) * TOP_K],
                            vals_sb[:, s * TOP_K:(s + 1) * TOP_K],
                            tk_sb[:, j * E:(j + 1) * E],
                        )
                # flush this pass's outputs so only the last pass's epilogue
                # sits in the kernel tail
                os_ = slice(p * GROUPS_PER_PASS * 4 * TOP_K,
                            (p + 1) * GROUPS_PER_PASS * 4 * TOP_K)
                nc.scalar.dma_start(w_out[:, os_], vals_sb[:, os_])
                nc.scalar.dma_start(i_out[:, os_], idx_sb[:, os_])

    _split_multi_waits(nc)
    return nc


def _get_program():
    global _PROGRAM
    if _PROGRAM is None:
        _PROGRAM = _build_program()
    return _PROGRAM


def _make_in_maps(x, proto_k, gate):
    xf = np.ascontiguousarray(x, dtype=np.float32).reshape(TOKENS, HIDDEN)
    proto = np.asarray(proto_k, dtype=np.float32)
    ph = proto.astype(np.float16)
    pl = ((proto - ph.astype(np.float32)) * LO_SCALE).astype(np.float16)
    phpl = np.concatenate([ph.T, pl.T], axis=1)           # [4096, 128] f16
    gate_neg = np.ascontiguousarray(
        -np.asarray(gate, dtype=np.float32).reshape(NUM_EXPERTS, 1)
    )
    in_maps = []
    for c in range(N_CORES):
        shard_t = xf[c * T_CORE:(c + 1) * T_CORE].T       # [4096, 2048] view
        xhl = np.empty((2, HIDDEN, T_CORE), np.float16)
        xhl[0] = shard_t
        xhl[1] = (shard_t - xhl[0].astype(np.float32)) * LO_SCALE
        in_maps.append(
            {"xhl": xhl, "phpl": phpl, "gate_neg": gate_neg}
        )
    return in_maps


def _gather(results):
    w = np.empty((TOKENS, TOP_K), np.float32)
    idx = np.empty((TOKENS, TOP_K), np.int32)
    for c in range(N_CORES):
        wo = results[c]["w_out"]                          # [128, 16*8]
        io = results[c]["i_out"].view(np.int32)
        w[c * T_CORE:(c + 1) * T_CORE] = (
            wo.reshape(128, N_SUB, TOP_K).transpose(1, 0, 2).reshape(T_CORE, TOP_K)
        )
        idx[c * T_CORE:(c + 1) * T_CORE] = (
            io.reshape(128, N_SUB, TOP_K).transpose(1, 0, 2).reshape(T_CORE, TOP_K)
        )
    return w.reshape(4, 4096, TOP_K), idx.reshape(4, 4096, TOP_K)


def run_sharded(in_maps, trace=False, trace_cores=None):
    from concourse.bass_utils import run_bass_kernel_spmd

    nc = _get_program()
    return run_bass_kernel_spmd(
        nc,
        in_maps,
        core_ids=list(range(N_CORES)),
        trace=trace,
        trace_cores=trace_cores,
    )


def kernel(x, proto_k, gate):
    in_maps = _make_in_maps(x, proto_k, gate)
    res = run_sharded(in_maps, trace=False)
    return _gather(res.results)


# revision 34
# speedup vs baseline: 1.0744x; 1.0673x over previous
"""MoE routing kernel for Trainium2 (8 NeuronCores, SPMD data-parallel).

Computes, for x [4, 4096, 4096] f32, proto_k [64, 4096] f32, gate [64] f32:
    logits = relu(x @ proto_k.T / sqrt(4096) - gate)        # [B, S, 64]
    routing_weights, selected_experts = top_k(logits, k=8)  # [B, S, 8] each

Sharding: tokens (B*S = 16384) are split evenly across 8 cores (2048 each).
proto_k / gate are replicated. No collectives needed.

Numerics: the matmul runs as a 3-term fp16 hi/lo split (x = xh + xl,
proto = ph + pl, logits = xh@ph + xh@pl + xl@ph, dropping xl@pl ~ 2^-22).
The residuals are pre-scaled by 2^11 on the host so they stay in fp16's
normal range, accumulated in a second PSUM bank, and recombined as
hi + 2^-11 * lo on the DVE.  Validated: bit-noise-level agreement with the
fp32 reference (max logit perturbation ~4e-8, zero top-8 index flips),
while streaming the PE at fp16 rate (1 cycle/row, 3 passes) instead of
fp32's 4 cycles/row with serialized weight loads.

Per-core device program:
  - x shard is split/transposed on the host to xh/xl [4096, 2048] fp16 so
    every DMA is contiguous and the contraction dim rides SBUF partitions.
  - logits accumulate with experts on partitions: per 128-wide hidden chunk,
    3 matmuls into 2 PSUM banks ([64, 512] per 512-token group).
  - DVE recombines hi + 2^-11*lo; ScalarE applies relu(acc/64 - gate).
  - TensorE transposes [64, 128] tiles -> [128 tokens, 64 experts] PSUM.
  - DVE Max8/MaxIndex emit top-8 values (descending) + indices per token.
  - Outputs pack as [128, 16*8] tiles, unscrambled on the host.
"""

import numpy as np

HIDDEN = 4096
NUM_EXPERTS = 64
TOP_K = 8
N_CORES = 8
TOKENS = 4 * 4096
T_CORE = TOKENS // N_CORES          # 2048 tokens per core
N_CHUNK = HIDDEN // 128             # 32 contraction chunks
GROUPS_PER_PASS = 2                 # 512-token groups accumulated per pass
N_PASS = T_CORE // (512 * GROUPS_PER_PASS)
N_SUB = T_CORE // 128               # 16 output sub-tiles of 128 tokens
LO_SCALE = np.float32(2.0 ** 11)
LO_UNSCALE = 2.0 ** -11

_PROGRAM = None


def _split_multi_waits(nc):
    """walrus in this container rejects instructions carrying more sync waits
    than their ISA struct holds (setupSyncWait: 'Too many sync wait
    commands'); Drain takes one, S3_LW (matmul weight-load) ~two.  Normalize
    every instruction to a single wait by hoisting extras onto same-engine
    NOPs inserted immediately before the owner."""
    import bass_rust

    inserts = {}  # owner inst name -> list of wait-nop instructions
    for f in nc.m.functions:
        for bb in f.blocks:
            for inst in bb.instructions:
                si = inst.sync_info
                if si is None or len(si.on_wait) <= 1:
                    continue
                conds = list(si.on_wait)
                si.on_wait = conds[:1]
                eng = nc.engines[inst.engine]
                new_insts = []
                for w in conds[1:]:
                    nop = eng.nop(hint="split_wait")
                    nop.ins.sync_info = bass_rust.SyncInfo(
                        on_wait=[w], on_update=[]
                    )
                    new_insts.append(nop.ins)
                inserts[inst.name] = new_insts
    if not inserts:
        return
    # nop() appended the new instructions to whatever bb was current; strip
    # them from everywhere, then re-insert each right before its owner so
    # the engine observes every wait before executing the instruction.
    appended = {ni.name for nis in inserts.values() for ni in nis}
    for f in nc.m.functions:
        for bb in f.blocks:
            rebuilt = []
            changed = False
            for inst in bb.instructions:
                if inst.name in appended:
                    changed = True
                    continue
                if inst.name in inserts:
                    rebuilt.extend(inserts[inst.name])
                    changed = True
                rebuilt.append(inst)
            if changed:
                bb.instructions = rebuilt


def _build_program():
    import concourse.bass as bass
    import concourse.mybir as mybir
    import concourse.tile as tile

    f32 = mybir.dt.float32
    f16 = mybir.dt.float16
    u32 = mybir.dt.uint32
    E = NUM_EXPERTS

    nc = bass.Bass("TRN2", target_bir_lowering=False, debug=False)

    # xh and xl stacked: xhl[0] = hi, xhl[1] = lo (one DMA fetches both)
    xhl_d = nc.dram_tensor("xhl", [2, HIDDEN, T_CORE], f16, kind="ExternalInput")
    # proto hi|lo packed along expert columns: [:, 0:64] = ph, [:, 64:128] = pl
    phpl_d = nc.dram_tensor("phpl", [HIDDEN, 2 * E], f16, kind="ExternalInput")
    gate_neg = nc.dram_tensor("gate_neg", [E, 1], f32, kind="ExternalInput")
    w_out = nc.dram_tensor("w_out", [128, N_SUB * TOP_K], f32, kind="ExternalOutput")
    i_out = nc.dram_tensor("i_out", [128, N_SUB * TOP_K], u32, kind="ExternalOutput")

    ident_dram = nc.inline_tensor(np.eye(E, dtype=np.float32), name="ident64")

    with tile.TileContext(nc) as tc:
        with (
            tc.tile_pool(name="const", bufs=1) as const_pool,
            tc.tile_pool(name="xa", bufs=12) as x_pool,
            tc.tile_pool(name="acc", bufs=7, space="PSUM") as acc_pool,
            tc.tile_pool(name="tp", bufs=1, space="PSUM") as tp_pool,
            tc.tile_pool(name="lg", bufs=3) as lg_pool,
            tc.tile_pool(name="tk", bufs=3) as tk_pool,
            tc.tile_pool(name="outp", bufs=1) as out_pool,
        ):
            # --- constants ---
            # proto chunks land as [128, c, E]; per-chunk DMAs are contiguous
            # 32 KB and let the first matmuls start early.
            # weights ride the (otherwise idle) gpsimd SWDGE ring so neither
            # the x stream (sync ring) nor the epilogue traffic (scalar
            # ring) queues behind their 32 triggers.
            phpl_sb = const_pool.tile([128, N_CHUNK * 2 * E], f16)
            for c in range(N_CHUNK):
                nc.gpsimd.dma_start(
                    phpl_sb[:, c * 2 * E:(c + 1) * 2 * E],
                    phpl_d[c * 128:(c + 1) * 128, :],
                )
            gate_sb = const_pool.tile([E, 1], f32)
            nc.scalar.dma_start(gate_sb[:], gate_neg[:])
            ident_sb = const_pool.tile([E, E], f32)
            nc.scalar.dma_start(ident_sb[:], ident_dram[:])

            vals_sb = out_pool.tile([128, N_SUB * TOP_K], f32)
            idx_sb = out_pool.tile([128, N_SUB * TOP_K], u32)

            for p in range(N_PASS):
                tpp = GROUPS_PER_PASS * 512
                t0 = p * tpp
                # a = xh @ [ph|pl]: rows 0:64 main term, 64:128 lo (2^11)
                # b = xl @ [ph|pl]: rows 0:64 lo (2^11), 64:128 llo (2^22)
                a_accs = [
                    acc_pool.tile([128, 512], f32, name=f"a_p{p}g{g}", tag="acc")
                    for g in range(GROUPS_PER_PASS)
                ]
                b_accs = [
                    acc_pool.tile([128, 512], f32, name=f"b_p{p}g{g}", tag="acc")
                    for g in range(GROUPS_PER_PASS)
                ]
                for c in range(N_CHUNK):
                    # one HWDGE DMA per chunk fetches hi and lo halves;
                    # alternate between the two HWDGE rings (SP / ACT) so
                    # trigger issue is never the bottleneck
                    x_t = x_pool.tile([128, 2, tpp], f16, name="x_t", tag="xt")
                    src = (xhl_d[:, c * 128:(c + 1) * 128, t0:t0 + tpp]
                           .rearrange("s p t -> p s t"))
                    if p == 0 and c == 0:
                        # split the very first chunk per group across both
                        # rings: the first matmul then waits on a 256 KB
                        # transfer instead of 512 KB
                        nc.sync.dma_start(x_t[:, :, 0:512], src[:, :, 0:512])
                        nc.scalar.dma_start(x_t[:, :, 512:tpp], src[:, :, 512:tpp])
                    else:
                        ring = nc.sync if c % 2 == 0 else nc.scalar
                        ring.dma_start(x_t[:], src)
                    first, last = (c == 0), (c == N_CHUNK - 1)
                    pc = slice(c * 2 * E, (c + 1) * 2 * E)
                    for g in range(GROUPS_PER_PASS):
                        ts = slice(g * 512, (g + 1) * 512)
                        nc.tensor.matmul(
                            a_accs[g][:], phpl_sb[:, pc], x_t[:, 0, ts],
                            start=first, stop=last,
                        )
                        nc.tensor.matmul(
                            b_accs[g][:], phpl_sb[:, pc], x_t[:, 1, ts],
                            start=first, stop=last,
                        )
                for g in range(GROUPS_PER_PASS):
                    # comb = a[0:64] + 2^-11*(a[64:128] + b[0:64] + 2^-11*b[64:128])
                    # DVE reads at most one PSUM input per op, so `a` is
                    # staged through SBUF (which also releases its PSUM bank
                    # for the next pass early).  The reads of the [64:128]
                    # halves into 0:64-partition outputs are cross-partition
                    # APs — verified exact on hardware.
                    a_sb = lg_pool.tile([128, 512], f32, name="a_sb")
                    nc.vector.tensor_copy(a_sb[:], a_accs[g][:])
                    u = lg_pool.tile([E, 512], f32, name="u")
                    nc.vector.scalar_tensor_tensor(
                        u[:], b_accs[g][0:E, :], 1.0, a_sb[E:2 * E, :],
                        bass.mybir.AluOpType.mult, bass.mybir.AluOpType.add,
                    )
                    v = lg_pool.tile([E, 512], f32, name="v")
                    nc.vector.scalar_tensor_tensor(
                        v[:], b_accs[g][E:2 * E, :], LO_UNSCALE, u[:],
                        bass.mybir.AluOpType.mult, bass.mybir.AluOpType.add,
                    )
                    comb = lg_pool.tile([E, 512], f32, name="comb")
                    nc.vector.scalar_tensor_tensor(
                        comb[:], v[:], LO_UNSCALE, a_sb[0:E, :],
                        bass.mybir.AluOpType.mult, bass.mybir.AluOpType.add,
                    )
                    # relu(acc/64 - gate)  (ScalarE, SBUF -> SBUF)
                    logits = lg_pool.tile([E, 512], f32, name="logits")
                    nc.scalar.activation(
                        logits[:], comb[:],
                        bass.mybir.ActivationFunctionType.Relu,
                        bias=gate_sb[:], scale=1.0 / 64.0,
                    )
                    # transpose to [128 tokens, 64 experts] x 4 sub-tiles
                    tk_psum = tp_pool.tile([128, 4 * E], f32, name="tk_psum")
                    for j in range(4):
                        nc.tensor.transpose(
                            tk_psum[:, j * E:(j + 1) * E],
                            logits[:, j * 128:(j + 1) * 128],
                            ident_sb[:],
                        )
                    tk_sb = tk_pool.tile([128, 4 * E], f32, name="tk_sb")
                    nc.vector.tensor_copy(tk_sb[:], tk_psum[:])
                    gg = p * GROUPS_PER_PASS + g
                    for j in range(4):
                        s = gg * 4 + j
                        nc.vector.max(
                            vals_sb[:, s * TOP_K:(s + 1) * TOP_K],
                            tk_sb[:, j * E:(j + 1) * E],
                        )
                        nc.vector.max_index(
                            idx_sb[:, s * TOP_K:(s + 1) * TOP_K],
                            vals_sb[:, s * TOP_K:(s + 1) * TOP_K],
                            tk_sb[:, j * E:(j + 1) * E],
                        )
                # flush this pass's outputs so only the last pass's epilogue
                # sits in the kernel tail
                os_ = slice(p * GROUPS_PER_PASS * 4 * TOP_K,
                            (p + 1) * GROUPS_PER_PASS * 4 * TOP_K)
                nc.scalar.dma_start(w_out[:, os_], vals_sb[:, os_])
                nc.scalar.dma_start(i_out[:, os_], idx_sb[:, os_])

    _split_multi_waits(nc)
    return nc


def _get_program():
    global _PROGRAM
    if _PROGRAM is None:
        _PROGRAM = _build_program()
    return _PROGRAM


def _make_in_maps(x, proto_k, gate):
    xf = np.ascontiguousarray(x, dtype=np.float32).reshape(TOKENS, HIDDEN)
    proto = np.asarray(proto_k, dtype=np.float32)
    ph = proto.astype(np.float16)
    pl = ((proto - ph.astype(np.float32)) * LO_SCALE).astype(np.float16)
    phpl = np.concatenate([ph.T, pl.T], axis=1)           # [4096, 128] f16
    gate_neg = np.ascontiguousarray(
        -np.asarray(gate, dtype=np.float32).reshape(NUM_EXPERTS, 1)
    )
    in_maps = []
    for c in range(N_CORES):
        shard_t = xf[c * T_CORE:(c + 1) * T_CORE].T       # [4096, 2048] view
        xhl = np.empty((2, HIDDEN, T_CORE), np.float16)
        xhl[0] = shard_t
        xhl[1] = (shard_t - xhl[0].astype(np.float32)) * LO_SCALE
        in_maps.append(
            {"xhl": xhl, "phpl": phpl, "gate_neg": gate_neg}
        )
    return in_maps


def _gather(results):
    w = np.empty((TOKENS, TOP_K), np.float32)
    idx = np.empty((TOKENS, TOP_K), np.int32)
    for c in range(N_CORES):
        wo = results[c]["w_out"]                          # [128, 16*8]
        io = results[c]["i_out"].view(np.int32)
        w[c * T_CORE:(c + 1) * T_CORE] = (
            wo.reshape(128, N_SUB, TOP_K).transpose(1, 0, 2).reshape(T_CORE, TOP_K)
        )
        idx[c * T_CORE:(c + 1) * T_CORE] = (
            io.reshape(128, N_SUB, TOP_K).transpose(1, 0, 2).reshape(T_CORE, TOP_K)
        )
    return w.reshape(4, 4096, TOP_K), idx.reshape(4, 4096, TOP_K)


def run_sharded(in_maps, trace=False, trace_cores=None):
    from concourse.bass_utils import run_bass_kernel_spmd

    nc = _get_program()
    return run_bass_kernel_spmd(
        nc,
        in_maps,
        core_ids=list(range(N_CORES)),
        trace=trace,
        trace_cores=trace_cores,
    )


def kernel(x, proto_k, gate):
    in_maps = _make_in_maps(x, proto_k, gate)
    res = run_sharded(in_maps, trace=False)
    return _gather(res.results)
